# revision 2
# baseline (speedup 1.0000x reference)
"""Block-wise embedding lookup on 8 Trainium2 NeuronCores — v7.

v5's two dma_gather classes (2-row pairs via an overlapping-window AP +
singles, vocab-parallel bf16, host routing/reassembly) with schedule
surgery from the v5/v6 traces:

- mlp library reload triggered as the FIRST Pool instruction (the ucode
  load blocks every later Pool instruction for ~9.5 us; triggering it at
  the head instead of mid-stream saves ~3 us);
- index uploads on the Sync engine so the Pool engine never waits on them;
- gather spans taper (8,8,...,2,1 groups) so the final gather->write->drain
  chain moves ~128 KiB instead of ~1 MiB;
- span writes alternate between the two HWDGE rings (Sync and Scalar
  engines) so the write backlog drains through two queues.
"""

import numpy as np
import ml_dtypes

N_CORES = 8
B, S, DIM, VOCAB = 32, 2048, 512, 100000
TOK = B * S
ROWS_PC = VOCAB // N_CORES
P = 128
BLOCK_OFFSETS = np.array([0, 50000, 80000, 95000], dtype=np.int32)

_CACHE = {}


def _spans(G, cap=8):
    """Contiguous spans of <= cap groups with a 2,1 taper at the end."""
    s = [(a, min(a + cap, G)) for a in range(0, G, cap)]
    if s and s[-1][1] - s[-1][0] > 3:
        a, b = s[-1]
        s[-1] = (a, b - 3)
        s.append((b - 3, b - 1))
        s.append((b - 1, b))
    return s


def _build_nc(PG, SG):
    from contextlib import ExitStack
    from concourse import bass, mybir
    from concourse import bacc, library_config

    WP, WS = PG * 2 * DIM, SG * DIM
    nc = bacc.Bacc("TRN2", target_bir_lowering=False)
    idx_d = nc.declare_dram_parameter(
        "idx", [P, (PG + SG) * 8], mybir.dt.int16, isOutput=False
    )
    slc = nc.declare_dram_parameter(
        "slice", [ROWS_PC, DIM], mybir.dt.bfloat16, isOutput=False
    )
    out = nc.declare_dram_parameter(
        "out", [P, WP + WS], mybir.dt.bfloat16, isOutput=True
    )

    pair_spans = _spans(PG)
    single_spans = _spans(SG)
    ng = len(pair_spans) + len(single_spans)

    with ExitStack() as ctx:
        block = ctx.enter_context(nc.Block(no_gpsimd_drain=True))
        s0 = ctx.enter_context(nc.semaphore("s0"))
        sem_g = [ctx.enter_context(nc.semaphore(f"sg{j}")) for j in range(ng)]
        sem_w = ctx.enter_context(nc.semaphore("sw"))
        sem_w2 = ctx.enter_context(nc.semaphore("sw2"))
        idx_t = ctx.enter_context(
            nc.sbuf_tensor("idx_t", [P, (PG + SG) * 8], mybir.dt.int16)
        )
        bufP = ctx.enter_context(
            nc.sbuf_tensor("bufP", [P, PG, 2 * DIM], mybir.dt.bfloat16)
        )
        bufS = ctx.enter_context(nc.sbuf_tensor("bufS", [P, SG, DIM], mybir.dt.bfloat16))
        bufP2 = bufP.reshape([P, WP])
        bufS2 = bufS.reshape([P, WS])

        base = slc[:, :]
        pair_view = bass.AP(base.tensor, 0, [(DIM, ROWS_PC - 1), (1, 2 * DIM)])

        # write j: (dram lo, dram hi, sbuf ap, sbuf lo, sbuf hi, gather sem)
        writes = []
        for j, (a, b) in enumerate(pair_spans):
            writes.append((a * 2 * DIM, b * 2 * DIM, 0, sem_g[j]))
        for j, (a, b) in enumerate(single_spans):
            writes.append((WP + a * DIM, WP + b * DIM, 1, sem_g[len(pair_spans) + j]))
        w_sync = writes[0::2]
        w_scal = writes[1::2]

        def emit_writes(eng, wlist, sem):
            for lo, hi, which, sg in wlist:
                src = bufP2 if which == 0 else bufS2
                off = 0 if which == 0 else WP
                eng.dma_start(
                    out=out[:, lo:hi], in_=src[:, lo - off : hi - off]
                )._wait_ge(sg, 16).then_inc(sem, 16)
            eng.wait_ge(sem, 16 * len(wlist))

        @block.sync
        def _(sync):
            sync.dma_start(out=idx_t[:, :], in_=idx_d[:, :]).then_inc(s0, 16)
            emit_writes(sync, w_sync, sem_w)

        @block.scalar
        def _(scalar):
            emit_writes(scalar, w_scal, sem_w2)

        @block.gpsimd
        def _(gpsimd):
            gpsimd.load_library(library_config.mlp)
            k = 0
            for a, b in pair_spans:
                n = (b - a) * P
                inst = gpsimd.dma_gather(
                    bufP[:, a:b, :],
                    pair_view,
                    idx_t[:, a * 8 : b * 8],
                    n,
                    n,
                    2 * DIM,
                    elem_step=DIM,
                ).then_inc(sem_g[k], 16)
                if k == 0:
                    inst._wait_ge(s0, 16)
                k += 1
            for a, b in single_spans:
                n = (b - a) * P
                gpsimd.dma_gather(
                    bufS[:, a:b, :],
                    slc[:, :],
                    idx_t[:, (PG + a) * 8 : (PG + b) * 8],
                    n,
                    n,
                    DIM,
                ).then_inc(sem_g[k], 16)
                k += 1

    nc.compile()
    return nc


def _get_nc(PG, SG):
    if (PG, SG) not in _CACHE:
        _CACHE[(PG, SG)] = _build_nc(PG, SG)
    return _CACHE[(PG, SG)]


def _route(src, block_assign, local_assign):
    ba = np.asarray(block_assign, np.int32).reshape(-1)
    la = np.asarray(local_assign, np.int32).reshape(-1)
    src_flat = np.asarray(src, np.int32).reshape(-1)
    gidx = BLOCK_OFFSETS[ba[src_flat]] + la[src_flat]
    shard = gidx // ROWS_PC
    order = np.argsort(shard, kind="stable")
    counts = np.bincount(shard, minlength=N_CORES)
    starts = np.concatenate([[0], np.cumsum(counts)[:-1]])
    loc = (gidx - shard * ROWS_PC).astype(np.int32)
    per_core = []
    for k in range(N_CORES):
        toks = order[starts[k] : starts[k] + counts[k]]
        rows = loc[toks]
        o2 = np.argsort(rows, kind="stable")
        toks, rows = toks[o2], rows[o2]
        n = len(rows)
        d = np.diff(rows) == 1
        is_pair = np.zeros(n, bool)
        i = 0
        while i < n - 1:
            if d[i]:
                is_pair[i] = True
                i += 2
            else:
                i += 1
        pi = np.where(is_pair)[0]
        taken = np.zeros(n, bool)
        taken[pi] = True
        taken[pi + 1] = True
        si = np.where(~taken)[0]
        per_core.append(
            {
                "pair_rows": rows[pi].astype(np.int16),
                "pair_toks": np.stack([toks[pi], toks[pi + 1]], 1)
                if len(pi)
                else np.zeros((0, 2), np.int64),
                "single_rows": rows[si].astype(np.int16),
                "single_toks": toks[si],
            }
        )
    return per_core


def _wrap16(vals, groups):
    cap = groups * P
    full = np.zeros(cap, np.int16)
    full[: len(vals)] = vals
    return np.tile(full.reshape(groups * 8, 16).T, (8, 1)).astype(np.int16)


def prepare_in_maps(src, block_assign, local_assign, table0, table1, table2, table3):
    big = np.concatenate(
        [
            np.asarray(t, dtype=np.float32).astype(ml_dtypes.bfloat16)
            for t in (table0, table1, table2, table3)
        ],
        axis=0,
    )
    assert big.shape == (VOCAB, DIM)
    per_core = _route(src, block_assign, local_assign)
    PG = max(1, max(-(-len(pc["pair_rows"]) // P) for pc in per_core))
    SG = max(1, max(-(-len(pc["single_rows"]) // P) for pc in per_core))
    in_maps = []
    for k, pc in enumerate(per_core):
        idx_arr = np.concatenate(
            [_wrap16(pc["pair_rows"], PG), _wrap16(pc["single_rows"], SG)], axis=1
        )
        in_maps.append(
            {
                "idx": np.ascontiguousarray(idx_arr),
                "slice": big[k * ROWS_PC : (k + 1) * ROWS_PC],
            }
        )
    return in_maps, per_core, PG, SG


def assemble_output(results, per_core, PG, SG):
    full = np.empty((TOK, DIM), np.float32)
    for r, pc in zip(results, per_core):
        o = np.asarray(r["out"])
        og = o.reshape(P, 2 * PG + SG, DIM)
        npair = len(pc["pair_rows"])
        if npair:
            u = np.arange(npair)
            p_, cg = u % P, (u // P) * 2
            full[pc["pair_toks"][:, 0]] = og[p_, cg].astype(np.float32)
            full[pc["pair_toks"][:, 1]] = og[p_, cg + 1].astype(np.float32)
        nsing = len(pc["single_rows"])
        if nsing:
            v = np.arange(nsing)
            p_, cg = v % P, 2 * PG + (v // P)
            full[pc["single_toks"]] = og[p_, cg].astype(np.float32)
    return full.reshape(B, S, DIM)


def prepare_run(np_inputs):
    in_maps, per_core, PG, SG = prepare_in_maps(**np_inputs)
    nc = _get_nc(PG, SG)

    def assemble(results):
        return assemble_output(results, per_core, PG, SG)

    return nc, in_maps, assemble


def kernel(src, block_assign, local_assign, table0, table1, table2, table3):
    from concourse.bass_utils import run_bass_kernel_spmd

    nc, in_maps, assemble = prepare_run(
        dict(
            src=src,
            block_assign=block_assign,
            local_assign=local_assign,
            table0=table0,
            table1=table1,
            table2=table2,
            table3=table3,
        )
    )
    res = run_bass_kernel_spmd(nc, in_maps, list(range(N_CORES)))
    return assemble(res.results)
